# revision 30
# baseline (speedup 1.0000x reference)
"""FKANLinear fused kernel v3 for 8 TRN2 NeuronCores.

Changes vs v2 baseline (engine rebalance; DVE was 72% busy, span 102us):
- fp16 planes/coefs everywhere (same matmul & DVE speed as bf16, 8x the
  mantissa).
- Fourier seeds without custom ISA: c2m = Sin(x/2 + [0;pi/2]) gives
  [sin(x/2); cos(x/2)] in one ACT op; q = s2*c2m; p1 = ts(q) -> [cos x;
  sin x]; cc = Square(c2m) on ACT; cdup = ts(cc) -> [cos x; cos x].
  ts (TensorScalar) runs in DVE 4x mode = 327ns/plane.
- Monomial chunks re-paired as P=[x;x^3] (1 ISA) and Q=TT(P,xd)=[x^2;x^4]
  (1 TT) instead of two ISA ops.
- Wavelet (u^2-1)e^{-u^2/2} split into two PE chunks: TT(u2,ew) and the
  ew plane itself with negated coefficients (PE has headroom; saves DVE).
- Single min/max reduce per super over a ts-built [-x; x] plane.
- Bias chunk folded into the spare bottom half of the 4th rho plane.
- Fourier chain TTs split DVE/Pool (tail of chain on Pool).
- y written by DMA straight from PSUM; xd copies via ACT (Copy is in
  every act table).
"""

import sys
import numpy as np

if "/opt/trn_rl_repo" not in sys.path:
    sys.path.insert(0, "/opt/trn_rl_repo")

# ---------------------------------------------------------------------------
# custom DVE op registration (runtime; table ships in the NEFF)
# ---------------------------------------------------------------------------

_REGISTERED = {}


def register_ops():
    global _REGISTERED
    if _REGISTERED:
        return _REGISTERED
    from concourse import dve_ops
    from concourse.dve_spec import (Spec, Src0, Src1, C0, C1, One, sq, relu,
                                    select, lower)
    from concourse.dve_spec import _has_src1 as has_src1
    from concourse.dve_uop import DveOpSpec

    s_ = sq(Src0)
    r_ = relu(Src0 + C0)
    defs = {
        # [x ; x^3] from xdup: s0 = 0 top / 1 bottom
        "ANT_FK_M13": Spec(
            body=select(C0, s_ * Src0, Src0),
            reference=lambda in0, in1, s0, s1, imm2: np.where(
                np.asarray(s0) != 0, in0 ** 3, in0),
        ),
        # relu(x + kappa)^3 (kappa = s0 per partition)
        "ANT_FK_RELUCUBE": Spec(
            body=sq(r_) * r_,
            reference=lambda in0, in1, s0, s1, imm2: np.maximum(
                in0 + np.asarray(s0), 0.0) ** 3,
        ),
    }

    base = max(dve_ops._SUB_OPCODE_FOR_NAME.values()) + 1
    for i, (name, spec) in enumerate(defs.items()):
        if name in dve_ops._SUB_OPCODE_FOR_NAME:
            continue
        opcode = base + i
        assert opcode < 0x20, "DVE opcode rows exhausted"
        dve_ops._SUB_OPCODE_FOR_NAME[name] = opcode
        shas = {}
        for ver in ("v3", "v4"):
            uops = lower(spec, ver=ver)
            shas[ver] = DveOpSpec(name=name, opcode=opcode, uops=uops,
                                  rd1_en=has_src1(spec)).sha(ver)
        op = dve_ops.DveOp(name, spec, subdim=False, uops_sha=shas)
        dve_ops.OPS.append(op)
        dve_ops.CUSTOM_DVE_SPECS[name] = spec
        _REGISTERED[name] = op
    if not _REGISTERED:
        by_name = {op.name: op for op in dve_ops.OPS}
        _REGISTERED = {n: by_name[n] for n in defs}
    return _REGISTERED


N_CORES = 8
B, IN, OUT = 32768, 64, 32
BS = B // N_CORES          # 4096 rows per core
SBC = 1024                 # batch columns per super-block
NSUP = BS // SBC           # 4
BC = 512                   # matmul moving-dim chunk
GRP = 512                  # rows per x-load DMA
G, P = 8, 3
TAY = 4
JDEG, JA, JB = 4, 1.0, 1.0
CDEG = 4
FREQ = 8
WCH = 4
TEMP = 2.0

F32 = np.float32

# ----------------------------------------------------------------------------
# host-side folding
# ----------------------------------------------------------------------------

def _softplus(z):
    z = np.asarray(z, np.float64)
    return np.log1p(np.exp(-np.abs(z))) + np.maximum(z, 0.0)


def _softmax(z, axis):
    z = np.asarray(z, np.float64)
    m = z.max(axis=axis, keepdims=True)
    e = np.exp(z - m)
    return e / e.sum(axis=axis, keepdims=True)


def _jacobi_mono():
    a, b = JA, JB
    terms = np.zeros((JDEG + 1, 5))
    terms[0, 0] = 1.0
    if JDEG >= 1:
        terms[1, 1] = 0.5 * 2.0 * (a + 1.0) / np.sqrt(2.0)
        terms[1, 0] = 0.5 * (a - b) / np.sqrt(2.0)
    for n in range(2, JDEG + 1):
        k = n - 1
        A1 = 2 * k + a + b
        A2 = 2 * (k + 1) * (k + a + b + 1) * (A1 + 1)
        A4 = 2 * (k + a) * (k + b) * (A1 + 2)
        c_x = (A1 + 1) * (A1 + 2) * A1 / A2
        c_0 = (A1 + 1) * (a * a - b * b) / A2
        Jn = np.zeros(5)
        Jn[1:] += c_x * terms[n - 1][:4]
        Jn += c_0 * terms[n - 1]
        Jn -= (A4 / A2) * terms[n - 2]
        terms[n] = Jn / np.sqrt(n + 1.0)
    return terms


def _cheby_mono():
    T = np.zeros((CDEG + 1, 5))
    T[0, 0] = 1.0
    T[1, 1] = 1.0
    for n in range(2, CDEG + 1):
        shift = np.zeros(5)
        shift[1:] = T[n - 1][:4]
        T[n] = 2.0 * shift - T[n - 2]
    norm = 1.0 / np.sqrt(np.arange(CDEG + 1) + 1.0)
    return T * norm[:, None]


def _bspline_tspace_phi(t):
    grid = np.concatenate([np.zeros(3), np.linspace(0.0, 8.0, G + 1), np.full(3, 8.0)])
    te = t[:, None]
    bases = ((te >= grid[None, :-1]) & (te < grid[None, 1:])).astype(np.float64)
    mask_last = t == grid[-1]
    bases[mask_last, :] = 0.0
    bases[mask_last, -1] = 1.0
    for r in range(1, P + 1):
        ld = np.maximum(grid[r:-1] - grid[:-(r + 1)], 1e-12)
        rd = np.maximum(grid[r + 1:] - grid[1:-r], 1e-12)
        left = (te - grid[None, :-(r + 1)]) / ld[None, :] * bases[:, :-1]
        right = (grid[None, r + 1:] - te) / rd[None, :] * bases[:, 1:]
        bases = left + right
    return bases


def _bspline_truncpow_matrix():
    S = 6000
    t = np.linspace(0.0, 8.0, S)
    t = t + 1e-7
    t = np.clip(t, 0.0, 8.0)
    phi = _bspline_tspace_phi(t)
    Fm = np.zeros((S, 11))
    Fm[:, 0] = 1.0
    Fm[:, 1] = t
    Fm[:, 2] = t * t
    Fm[:, 3] = t ** 3
    for j in range(1, 8):
        Fm[:, 3 + j] = np.maximum(t - j, 0.0) ** 3
    M, _, _, _ = np.linalg.lstsq(Fm, phi, rcond=None)
    return M


NCH = 14   # static chunk coef columns: P,Q, f1..f8, wA0,wA1, wB0,wB1


def fold_constants(inputs):
    base_v = np.asarray(inputs["base_v"], np.float64)
    base_g = np.asarray(inputs["base_g"], np.float64)
    base_bias = np.asarray(inputs["base_bias"], np.float64)
    gains = np.asarray(inputs["gains"], np.float64)
    alpha = float(_softplus(inputs["alpha_logit"]))
    beta = float(_softplus(inputs["beta_logit"]))
    mixw = _softmax(np.asarray(inputs["mix_logits"], np.float64) / TEMP, axis=-1)
    sg = _softplus(gains)

    def ceff(name, f):
        return np.asarray(inputs[name], np.float64) * mixw[..., f:f + 1] * sg[f] * beta

    C_bs = ceff("bspline_coef", 0)
    C_ty = ceff("taylor_coef", 1)
    C_jb = ceff("jacobi_coef", 2)
    C_cb = ceff("cheby_coef", 3)
    C_fr = ceff("fourier_coef", 4)
    C_wv = ceff("wavelet_coef", 5)

    vn = np.sqrt((base_v ** 2).sum(axis=1, keepdims=True))
    Walpha = alpha * base_g * base_v / vn
    bias_alpha = alpha * base_bias

    mono = np.zeros((OUT, IN, 5))
    fac = np.array([1.0, 1.0, 2.0, 6.0])
    mono[:, :, :4] += C_ty / fac[None, None, :]
    mono += np.einsum("oin,nd->oid", C_jb, _jacobi_mono())
    mono += np.einsum("oin,nd->oid", C_cb, _cheby_mono())

    fnorm = 1.0 / np.sqrt(2.0 * FREQ)
    Ccos = C_fr[:, :, :FREQ] * fnorm
    Csin = C_fr[:, :, FREQ:] * fnorm
    Tc = np.zeros((9, 9)); Tc[0, 0] = 1.0; Tc[1, 1] = 1.0
    Uc = np.zeros((9, 9)); Uc[0, 0] = 1.0; Uc[1, 1] = 2.0
    for n in range(2, 9):
        for M_ in (Tc, Uc):
            sh = np.zeros(9); sh[1:] = M_[n - 1][:8]
            M_[n] = 2.0 * sh - M_[n - 2]
    Ccpow = np.einsum("oik,kj->oij", Ccos, Tc[1:9, :])
    Cspow = np.einsum("oik,kj->oij", Csin, Uc[0:8, :])

    a_w = _softplus(np.asarray(inputs["wavelet_scale_logit"], np.float64)) + 1e-6
    inva = 1.0 / a_w
    nshia = -np.asarray(inputs["wavelet_shift"], np.float64) * inva

    M = _bspline_truncpow_matrix()
    CF = np.einsum("oik,mk->oim", C_bs, M)
    P_poly = CF[:, :, :4]
    R_rho = CF[:, :, 4:]

    # static chunk coefs: (128, NCH*32), fp16 on device
    CW = np.zeros((128, NCH * OUT))
    def put(c, half, arr_oi):
        CW[half * 64:(half + 1) * 64, 32 * c:32 * (c + 1)] = arr_oi.T
    # c0: P=[x; x^3], c1: Q=[x^2; x^4]
    put(0, 0, Walpha + mono[:, :, 1])
    put(0, 1, mono[:, :, 3])
    put(1, 0, mono[:, :, 2])
    put(1, 1, mono[:, :, 4])
    # c2..c9: fourier powers [cos^j; sin*cos^(j-1)]
    for j in range(1, 9):
        put(1 + j, 0, Ccpow[:, :, j])
        put(1 + j, 1, Cspow[:, :, j - 1])
    # c10,c11: wavelet A chunks (u2*ew); c12,c13: ew chunks (coef = -A)
    put(10, 0, C_wv[:, :, 0]); put(10, 1, C_wv[:, :, 1])
    put(11, 0, C_wv[:, :, 2]); put(11, 1, C_wv[:, :, 3])
    put(12, 0, -C_wv[:, :, 0]); put(12, 1, -C_wv[:, :, 1])
    put(13, 0, -C_wv[:, :, 2]); put(13, 1, -C_wv[:, :, 3])

    CO = (mono[:, :, 0] + Ccpow[:, :, 0]).T.copy()
    CO[0, :] += bias_alpha

    PW = np.zeros((128, 4 * OUT))
    for d in range(4):
        PW[0:64, 32 * d:32 * (d + 1)] = P_poly[:, :, d].T
    PW[64:128] = PW[0:64]
    RW = np.zeros((128, 7 * OUT))
    for j in range(7):
        RW[0:64, 32 * j:32 * (j + 1)] = R_rho[:, :, j].T
    RW[64:128] = RW[0:64]

    WVP = np.zeros((128, 4))
    for p in range(2):
        WVP[0:64, 2 * p] = inva[:, 2 * p]
        WVP[64:128, 2 * p] = inva[:, 2 * p + 1]
        WVP[0:64, 2 * p + 1] = nshia[:, 2 * p]
        WVP[64:128, 2 * p + 1] = nshia[:, 2 * p + 1]

    ID = np.eye(128)

    return {
        "CW": CW.astype(F32), "CO": CO.astype(F32), "PW": PW.astype(F32),
        "RW": RW.astype(F32), "WVP": WVP.astype(F32), "ID": ID.astype(F32),
    }


# ----------------------------------------------------------------------------
# numpy emulation (validates folding; mirrors device chunk structure)
# ----------------------------------------------------------------------------

def numpy_forward(inputs):
    consts = fold_constants(inputs)
    x = np.asarray(inputs["x"], np.float64)
    CW = consts["CW"].astype(np.float64)
    CO = consts["CO"].astype(np.float64)
    PW = consts["PW"].astype(np.float64)
    RW = consts["RW"].astype(np.float64)
    WVP = consts["WVP"].astype(np.float64)

    xmin = x.min(axis=0); xmax = x.max(axis=0)
    pad = (xmax - xmin) < 1e-8
    xmin = np.where(pad, xmin - 0.5, xmin)
    xmax = np.where(pad, xmax + 0.5, xmax)
    rng = xmax - xmin
    b = 8.0 / rng
    a = -xmin * b
    P_poly = np.stack([PW[0:64, 32 * d:32 * (d + 1)] for d in range(4)], axis=-1)
    binom = {(0, 0): 1, (1, 0): 1, (1, 1): 1, (2, 0): 1, (2, 1): 2, (2, 2): 1,
             (3, 0): 1, (3, 1): 3, (3, 2): 3, (3, 3): 1}
    Cdyn = np.zeros((IN, OUT, 4))
    for d in range(4):
        for e in range(d + 1):
            Cdyn[:, :, e] += P_poly[:, :, d] * (binom[(d, e)] * a ** (d - e) * b ** e)[:, None]
    # dyn-mono chunk coefs for P=[x;x^3] and Q=[x^2;-] planes
    CDP = np.zeros((128, OUT)); CDQ = np.zeros((64, OUT))
    CDP[0:64] = Cdyn[:, :, 1]; CDP[64:128] = Cdyn[:, :, 3]
    CDQ[0:64] = Cdyn[:, :, 2]
    bias = (CO + Cdyn[:, :, 0]).sum(axis=0)
    # rho chunk coefs; col 3 bottom carries the bias row-block (vs ones plane)
    RHW = np.zeros((128, 4 * OUT))
    for j in range(1, 8):
        q, half = (j - 1) // 2, (j - 1) % 2
        RHW[half * 64:(half + 1) * 64, 32 * q:32 * (q + 1)] = \
            RW[0:64, 32 * (j - 1):32 * j] * (b ** 3)[:, None]
    RHW[64:128, 96:128] = (CO + Cdyn[:, :, 0])

    Bn = x.shape[0]
    y = np.zeros((Bn, OUT))
    kap = [xmin + j * rng / 8.0 for j in range(1, 8)]

    feats = []
    # P, Q monomials
    f = np.zeros((Bn, 128)); f[:, 0:64] = x; f[:, 64:128] = x ** 3
    feats.append((f, CW[:, 0:32], 128))
    f = np.zeros((Bn, 128)); f[:, 0:64] = x * x; f[:, 64:128] = x ** 4
    feats.append((f, CW[:, 32:64], 128))
    # fourier powers
    cc_, ss_ = np.cos(x), np.sin(x)
    for j in range(1, 9):
        f = np.zeros((Bn, 128))
        f[:, 0:64] = cc_ ** j; f[:, 64:128] = ss_ * cc_ ** (j - 1)
        feats.append((f, CW[:, 32 * (1 + j):32 * (2 + j)], 128))
    # wavelets: A = u2*ew, B = ew
    for p in range(2):
        u0 = x * WVP[None, 0:64, 2 * p] + WVP[None, 0:64, 2 * p + 1]
        u1 = x * WVP[None, 64:128, 2 * p] + WVP[None, 64:128, 2 * p + 1]
        e0 = np.exp(-0.5 * u0 ** 2); e1 = np.exp(-0.5 * u1 ** 2)
        f = np.zeros((Bn, 128))
        f[:, 0:64] = u0 ** 2 * e0; f[:, 64:128] = u1 ** 2 * e1
        feats.append((f, CW[:, 32 * (10 + p):32 * (11 + p)], 128))
        f = np.zeros((Bn, 128))
        f[:, 0:64] = e0; f[:, 64:128] = e1
        feats.append((f, CW[:, 32 * (12 + p):32 * (13 + p)], 128))
    # rho (col 3: knot 7 top, ones bottom for bias)
    for q in range(4):
        j0 = 2 * q + 1
        f = np.zeros((Bn, 128))
        f[:, 0:64] = np.maximum(x - kap[j0 - 1][None, :], 0.0) ** 3
        if j0 + 1 <= 7:
            f[:, 64:128] = np.maximum(x - kap[j0][None, :], 0.0) ** 3
        else:
            f[:, 64:128] = 1.0
        feats.append((f, RHW[:, 32 * q:32 * (q + 1)], 128))
    # dyn-mono
    f = np.zeros((Bn, 128)); f[:, 0:64] = x; f[:, 64:128] = x ** 3
    feats.append((f, CDP, 128))
    f = np.zeros((Bn, 128)); f[:, 0:64] = x * x
    feats.append((f, CDQ, 64))

    for f, w, rows in feats:
        y = y + f[:, :rows] @ w[:rows]
    return y.astype(F32)


# ----------------------------------------------------------------------------
# device kernel
# ----------------------------------------------------------------------------

def build_nc(reps=1, no_collective=False, phases='full', n_pool_f=0,
             y_dma=True, plane_fp16=True, pool_q=False, skip_rho=False,
             nf=8, skip_wav=False):
    import concourse.bass as bass
    import concourse.bacc as bacc
    import concourse.mybir as mybir
    import concourse.tile as tile

    OPS = register_ops()

    dt = mybir.dt.float32
    hf = mybir.dt.float16 if plane_fp16 else mybir.dt.bfloat16
    AF = mybir.ActivationFunctionType
    ALU = mybir.AluOpType
    AX = mybir.AxisListType

    nc = bacc.Bacc("TRN2", target_bir_lowering=False, debug=False,
                   enable_asserts=True, num_devices=N_CORES)

    xs = nc.dram_tensor("xs", [BS, 2 * IN], mybir.dt.float16,
                        kind="ExternalInput").ap()
    cw_d = nc.dram_tensor("CW", [128, NCH * OUT], dt, kind="ExternalInput").ap()
    co_d = nc.dram_tensor("CO", [IN, OUT], dt, kind="ExternalInput").ap()
    pw_d = nc.dram_tensor("PW", [128, 4 * OUT], dt, kind="ExternalInput").ap()
    rw_d = nc.dram_tensor("RW", [128, 7 * OUT], dt, kind="ExternalInput").ap()
    wv_d = nc.dram_tensor("WVP", [128, 4], dt, kind="ExternalInput").ap()
    id_d = nc.dram_tensor("ID", [128, 128], dt, kind="ExternalInput").ap()
    y_d = nc.dram_tensor("y", [OUT, BS], dt, kind="ExternalOutput").ap()

    def cw(c):
        return cwt[:, 32 * c:32 * (c + 1)]

    with tile.TileContext(nc) as tc:
        with (
            tc.tile_pool(name="const", bufs=1) as cpool,
            tc.tile_pool(name="plane", bufs=8) as plane,
            tc.tile_pool(name="seed", bufs=4) as seedp,
            tc.tile_pool(name="cdw", bufs=4) as cdwp,
            tc.tile_pool(name="ytp", bufs=2) as ytp,
            tc.tile_pool(name="xpipe", bufs=3) as xpipe,
            tc.tile_pool(name="pers", bufs=1) as pers,
            tc.tile_pool(name="xdp", bufs=2) as xdp,
            tc.tile_pool(name="ps", bufs=2, space="PSUM") as ps,
            tc.tile_pool(name="psacc", bufs=3, space="PSUM") as psacc,
            tc.tile_pool(name="dram", bufs=1, space="DRAM") as dram,
        ):
            # ---- constants ----
            cwf = cpool.tile([128, NCH * OUT], dt, tag="cwf")
            cot = cpool.tile([IN, OUT], dt, tag="cot")
            pwt = cpool.tile([128, 4 * OUT], dt, tag="pwt")
            rwt = cpool.tile([128, 7 * OUT], dt, tag="rwt")
            wvt = cpool.tile([128, 4], dt, tag="wvt")
            idt = cpool.tile([128, 128], dt, tag="idt")
            idt16 = cpool.tile([128, 128], mybir.dt.float16, tag="idt16")
            nc.sync.dma_start(out=cwf[:, :], in_=cw_d[:, :])
            nc.sync.dma_start(out=cot[:, :], in_=co_d[:, :])
            nc.sync.dma_start(out=pwt[:, :], in_=pw_d[:, :])
            nc.sync.dma_start(out=rwt[:, :], in_=rw_d[:, :])
            nc.sync.dma_start(out=wvt[:, :], in_=wv_d[:, :])
            nc.sync.dma_start(out=idt[:, :], in_=id_d[:, :])
            cwt = cpool.tile([128, NCH * OUT], hf, tag="cwt")
            nc.vector.tensor_copy(out=cwt[:, :], in_=cwf[:, :])
            nc.vector.tensor_copy(out=idt16[:, :], in_=idt[:, :])

            # per-partition constant columns
            selneg = cpool.tile([128, 1], dt, tag="selneg")   # [-1;+1]
            nc.vector.memset(selneg[0:64, :], -1.0)
            nc.vector.memset(selneg[64:128, :], 1.0)
            selpi2 = cpool.tile([128, 1], dt, tag="selpi2")   # [0; pi/2]
            nc.vector.memset(selpi2[0:64, :], 0.0)
            nc.vector.memset(selpi2[64:128, :], float(np.pi / 2))
            selm2p2 = cpool.tile([128, 1], dt, tag="selm2p2")  # [-2; +2]
            nc.vector.memset(selm2p2[0:64, :], -2.0)
            nc.vector.memset(selm2p2[64:128, :], 2.0)
            sel10 = cpool.tile([128, 1], dt, tag="sel10")      # [1; 0]
            nc.vector.memset(sel10[0:64, :], 1.0)
            nc.vector.memset(sel10[64:128, :], 0.0)
            sel1m1 = cpool.tile([128, 1], dt, tag="sel1m1")    # [1; -1]
            nc.vector.memset(sel1m1[0:64, :], 1.0)
            nc.vector.memset(sel1m1[64:128, :], -1.0)
            sel01 = cpool.tile([128, 1], dt, tag="sel01")      # [0; 1]
            nc.vector.memset(sel01[0:64, :], 0.0)
            nc.vector.memset(sel01[64:128, :], 1.0)
            selhalf = cpool.tile([128, 1], dt, tag="selhalf")  # 0.5 everywhere
            nc.vector.memset(selhalf[:, :], 0.5)

            # rho plane q=3: bottom half is a persistent all-ones block
            # (bias chunk rides on it); top half rewritten per rep/super.
            rq3s = []
            for s in range(NSUP):
                t_ = cpool.tile([128, SBC], hf, tag=f"rq3_{s}")
                nc.vector.memset(t_[64:128, :], 1.0)
                rq3s.append(t_)

            state = {}

            def phase_a_all():
                # =========== phase A (all supers of one rep) ==============
                mm = xdp.tile([128, NSUP], dt, tag="mm")
                xds = [None] * NSUP
                state["mm"] = mm
                state["xds"] = xds

                def phase_a(s):
                    xd = xdp.tile([128, SBC], hf, tag=f"xd{s}")
                    xds[s] = xd
                    # one DMA per super; xs is host-duplicated [x|x] so each
                    # partition gets 8 consecutive rows = 4KB contiguous
                    # segments. xd column g*512+t*128+p holds batch row
                    # base+8p+4g+t; the y copy undoes the permutation.
                    base = s * SBC
                    xt8 = xpipe.tile([128, 16 * IN], mybir.dt.float16,
                                     tag="xin")
                    xq = nc.sync if s % 2 == 0 else nc.gpsimd
                    xq.dma_start(
                        out=xt8[:, :],
                        in_=xs[base:base + SBC, :].rearrange(
                            "(p t) i -> p (t i)", p=128))
                    for g in range(2):
                        tp = ps.tile([128, 512], mybir.dt.float16,
                                     tag="tp")
                        for t in range(4):
                            tt = 4 * g + t
                            nc.tensor.transpose(
                                tp[:, 128 * t:128 * (t + 1)],
                                xt8[:, tt * 128:(tt + 1) * 128], idt16[:, :])
                        nc.scalar.copy(xd[:, g * 512:(g + 1) * 512], tp[:, :])
                    # [-x; x] then one max-reduce -> [-min; max] per column
                    ng = seedp.tile([128, SBC], hf, tag="ng")
                    nc.vector.tensor_scalar(out=ng[:, :], in0=xd[:, :],
                                            scalar1=selneg[:, 0:1],
                                            scalar2=None, op0=ALU.mult)
                    nc.vector.tensor_reduce(out=mm[:, s:s + 1], in_=ng[:, :],
                                            axis=AX.X, op=ALU.max)

                for s in range(NSUP):
                    phase_a(s)

            phase_a_all()
            for _rep in range(reps):
                mm = state["mm"]
                xds = state["xds"]
                locmm = xdp.tile([128, 1], dt, tag="locmm")
                def phase_coll():
                    nc.vector.tensor_reduce(out=locmm[:, 0:1], in_=mm[:, 0:NSUP],
                                            axis=AX.X, op=ALU.max)
                    return _coll_body()

                # ====== collective: all-reduce-max per-column [-min;max] ==
                def _coll_body():
                    bounce_in = dram.tile([IN, 2], dt, tag="cin")
                    bounce_out = dram.tile([IN, 2], dt, tag="cout")
                    nc.sync.dma_start(
                        out=bounce_in.rearrange("i h -> h i"),
                        in_=locmm[:, 0:1].rearrange("p x -> (p x)"))
                    if no_collective:
                        nc.sync.dma_start(out=bounce_out[:, :],
                                          in_=bounce_in[:, :])
                    else:
                        nc.gpsimd.collective_compute(
                            "AllReduce", mybir.AluOpType.max,
                            replica_groups=[list(range(N_CORES))],
                            ins=[bounce_in.opt()],
                            outs=[bounce_out.opt()],
                        )
                    # gm[:,0] = -gmin, gm[:,1] = gmax on every partition
                    gm = xdp.tile([128, 2], dt, tag="gm")
                    nc.sync.dma_start(out=gm[0:IN, :], in_=bounce_out[:, :])
                    nc.sync.dma_start(out=gm[IN:128, :], in_=bounce_out[:, :])
                    return gm

                # =========== phase B per super (chunks 0..9) ==============
                accs = {}
                Ps = {}
                Qs = {}
                bdefer = {}
                wdefer = {}

                def mmc_on(acc, lhs, F, first=False, rows=128,
                           stop=False):
                    nc.tensor.matmul(acc[:, 0:BC], lhs[0:rows, :],
                                     F[0:rows, 0:BC], start=first, stop=stop)
                    nc.tensor.matmul(acc[:, BC:SBC], lhs[0:rows, :],
                                     F[0:rows, BC:SBC], start=first, stop=stop)

                def phase_b(s, defer=False):
                    xsl = xds[s][:, :]
                    if defer:
                        chunks = bdefer.setdefault(s, [])
                        emit = lambda c, F: chunks.append((c, F))
                    else:
                        acc = psacc.tile([OUT, SBC], dt, tag="acc")
                        accs[s] = acc
                        emit = lambda c, F: mmc_on(acc, cw(c), F, first=(c == 2))
                    ptag = "pl3" if defer else "pl"
                    pbufs = 8

                    # monomial planes P=[x;x^3] (ISA), Q=[x^2;x^4]=P*xd (TT)
                    Pp = xdp.tile([128, SBC], hf, tag=f"P_{s}")
                    Ps[s] = Pp
                    nc.vector._custom_dve(OPS["ANT_FK_M13"], out=Pp[:, :],
                                          in0=xsl, s0=sel01[:, 0:1])
                    Qp = xdp.tile([128, SBC], hf, tag=f"Q_{s}")
                    Qs[s] = Qp
                    qeng = nc.gpsimd if pool_q else nc.vector
                    qeng.tensor_tensor(out=Qp[:, :], in0=Pp[:, :],
                                       in1=xsl, op=ALU.mult)

                    # fourier seeds: s2=[sin(x/2)]², c2m=[sin(x/2); cos(x/2)]
                    s2 = seedp.tile([128, SBC], hf, tag="s2")
                    nc.scalar.activation(s2[:, :], xsl, AF.Sin, scale=0.5)
                    c2m = seedp.tile([128, SBC], hf, tag="c2m")
                    nc.scalar.activation(c2m[:, :], xsl, AF.Sin, scale=0.5,
                                         bias=selpi2[:, 0:1])
                    # cc = Square(c2m) = [sin²(x/2); cos²(x/2)]  (ACT)
                    ccp = cdwp.tile([128, SBC], hf, tag="ccp")
                    nc.scalar.activation(ccp[:, :], c2m[:, :], AF.Square)
                    # q = [sin²(x/2); sin(x/2)cos(x/2)]
                    qt = seedp.tile([128, SBC], hf, tag="qt")
                    nc.vector.tensor_tensor(out=qt[:, :], in0=s2[:, :],
                                            in1=c2m[:, :], op=ALU.mult)
                    # p1 = q*[-2;2] + [1;0] = [cos x; sin x]
                    pk = plane.tile([128, SBC], hf, tag=ptag, bufs=pbufs)
                    nc.vector.tensor_scalar(out=pk[:, :], in0=qt[:, :],
                                            scalar1=selm2p2[:, 0:1],
                                            scalar2=sel10[:, 0:1],
                                            op0=ALU.mult, op1=ALU.add)
                    # cdup = cc*[-2;2] + [1;-1] = [cos x; cos x]
                    cdup = cdwp.tile([128, SBC], hf, tag="cdup")
                    nc.vector.tensor_scalar(out=cdup[:, :], in0=ccp[:, :],
                                            scalar1=selm2p2[:, 0:1],
                                            scalar2=sel1m1[:, 0:1],
                                            op0=ALU.mult, op1=ALU.add)
                    emit(2, pk)
                    for j in range(2, nf + 1):
                        pn = plane.tile([128, SBC], hf, tag=ptag, bufs=pbufs)
                        eng = nc.gpsimd if j > 8 - n_pool_f else nc.vector
                        eng.tensor_tensor(out=pn[:, :], in0=pk[:, :],
                                          in1=cdup[:, :], op=ALU.mult)
                        emit(1 + j, pn)
                        pk = pn

                def phase_b_mm(s):
                    acc = psacc.tile([OUT, SBC], dt, tag="acc")
                    accs[s] = acc
                    for c, F in bdefer.pop(s):
                        mmc_on(acc, cw(c), F, first=(c == 2))

                def phase_b_wav(s, defer=False):
                    if skip_wav:
                        wdefer.setdefault(s, [])
                        return
                    xsl = xds[s][:, :]
                    if defer:
                        chunks = wdefer.setdefault(s, [])
                        emit = lambda c, F: chunks.append((c, F))
                    else:
                        acc = accs[s]
                        emit = lambda c, F: mmc_on(acc, cw(c), F)
                    ptag = "pl3w" if defer else "pl"

                    for p in range(2):
                        u2 = cdwp.tile([128, SBC], hf, tag="u2", bufs=6)
                        nc.scalar.activation(u2[:, :], xsl, AF.Square,
                                             bias=wvt[:, 2 * p + 1:2 * p + 2],
                                             scale=wvt[:, 2 * p:2 * p + 1])
                        ew = cdwp.tile([128, SBC], hf, tag="ew", bufs=6)
                        nc.scalar.activation(ew[:, :], u2[:, :], AF.Exp,
                                             scale=-0.5)
                        wf = plane.tile([128, SBC], hf, tag=ptag,
                                        bufs=(2 if defer else 8))
                        nc.vector.tensor_tensor(out=wf[:, :], in0=u2[:, :],
                                                in1=ew[:, :], op=ALU.mult)
                        emit(10 + p, wf)
                        emit(12 + p, ew)

                def phase_b_wav_mm(s):
                    acc = accs[s]
                    for c, F in wdefer.pop(s):
                        mmc_on(acc, cw(c), F)

                # =========== phase C: post-collective remix ================
                def phase_c(gm):
                    v = pers.tile([128, 24], dt, tag="vecs")
                    rng_, msk = v[:, 2:3], v[:, 3:4]
                    gmn2, gmax2, rng2 = v[:, 4:5], v[:, 5:6], v[:, 6:7]
                    rinv, bb, aa = v[:, 7:8], v[:, 8:9], v[:, 9:10]
                    b2, b3, a2, a3 = v[:, 10:11], v[:, 11:12], v[:, 12:13], v[:, 13:14]
                    ab, a2b, ab2, rstep = (v[:, 14:15], v[:, 15:16], v[:, 16:17],
                                           v[:, 17:18])
                    # gm[:,0] = -gmin, gm[:,1] = gmax (already reduced)
                    gmn, gmax = gm[:, 0:1], gm[:, 1:2]
                    # rng = gmax + gmn  (gmn = -gmin)
                    nc.vector.tensor_tensor(out=rng_[:, :], in0=gmax[:, :],
                                            in1=gmn[:, :], op=ALU.add)
                    nc.vector.tensor_scalar(out=msk[:, :], in0=rng_[:, :],
                                            scalar1=1e-8, scalar2=0.5,
                                            op0=ALU.is_lt, op1=ALU.mult)
                    # gmn2 = -gmin2 = gmn + msk ;  gmax2 = gmax + msk
                    nc.vector.tensor_tensor(out=gmn2[:, :], in0=gmn[:, :],
                                            in1=msk[:, :], op=ALU.add)
                    nc.vector.tensor_tensor(out=gmax2[:, :], in0=gmax[:, :],
                                            in1=msk[:, :], op=ALU.add)
                    nc.vector.tensor_tensor(out=rng2[:, :], in0=gmax2[:, :],
                                            in1=gmn2[:, :], op=ALU.add)
                    nc.vector.reciprocal(out=rinv[:, :], in_=rng2[:, :])
                    nc.vector.tensor_scalar_mul(out=bb[:, :], in0=rinv[:, :],
                                                scalar1=8.0)
                    # aa = -gmin*b = gmn2*bb
                    nc.vector.tensor_tensor(out=aa[:, :], in0=gmn2[:, :],
                                            in1=bb[:, :], op=ALU.mult)
                    nc.vector.tensor_tensor(out=b2[:, :], in0=bb[:, :], in1=bb[:, :],
                                            op=ALU.mult)
                    nc.vector.tensor_tensor(out=b3[:, :], in0=b2[:, :], in1=bb[:, :],
                                            op=ALU.mult)
                    nc.vector.tensor_tensor(out=a2[:, :], in0=aa[:, :], in1=aa[:, :],
                                            op=ALU.mult)
                    nc.vector.tensor_tensor(out=a3[:, :], in0=a2[:, :], in1=aa[:, :],
                                            op=ALU.mult)
                    nc.vector.tensor_tensor(out=ab[:, :], in0=aa[:, :], in1=bb[:, :],
                                            op=ALU.mult)
                    nc.vector.tensor_tensor(out=a2b[:, :], in0=a2[:, :], in1=bb[:, :],
                                            op=ALU.mult)
                    nc.vector.tensor_tensor(out=ab2[:, :], in0=aa[:, :], in1=b2[:, :],
                                            op=ALU.mult)
                    nc.vector.tensor_scalar_mul(out=rstep[:, :], in0=rng2[:, :],
                                                scalar1=0.125)

                    # kappa_j = gmin2 + j*rng/8  ->  kp = -kappa = gmn2 - j*rstep
                    kn = pers.tile([128, 7], dt, tag="kn")
                    for j in range(1, 8):
                        nc.vector.scalar_tensor_tensor(
                            out=kn[:, j - 1:j], in0=rstep[:, :], scalar=-float(j),
                            in1=gmn2[:, :], op0=ALU.mult, op1=ALU.add)
                    kp = pers.tile([128, 4], dt, tag="kp")
                    for q in range(4):
                        nc.vector.tensor_copy(out=kp[0:IN, q:q + 1],
                                              in_=kn[0:IN, 2 * q:2 * q + 1])
                        if 2 * q + 1 < 7:
                            nc.vector.tensor_copy(out=kp[IN:128, q:q + 1],
                                                  in_=kn[IN:128, 2 * q + 1:2 * q + 2])

                    # dynamic monomial chunk coefs (for P and Q planes)
                    cdpf = pers.tile([128, OUT], dt, tag="cdpf")
                    cdqf = pers.tile([64, OUT], dt, tag="cdqf")
                    cd0 = pers.tile([128, OUT], dt, tag="cd0")
                    tmp = pers.tile([128, OUT], dt, tag="cdtmp")
                    P0, P1 = pwt[:, 0:32], pwt[:, 32:64]
                    P2, P3 = pwt[:, 64:96], pwt[:, 96:128]
                    # cd0 = P0 + a*P1 + a^2*P2 + a^3*P3   (const plane)
                    nc.vector.tensor_scalar(out=cd0[:, :], in0=P1, scalar1=aa[:, 0:1],
                                            scalar2=None, op0=ALU.mult)
                    nc.vector.tensor_tensor(out=cd0[:, :], in0=cd0[:, :], in1=P0,
                                            op=ALU.add)
                    nc.vector.tensor_scalar(out=tmp[:, :], in0=P2, scalar1=a2[:, 0:1],
                                            scalar2=None, op0=ALU.mult)
                    nc.vector.tensor_tensor(out=cd0[:, :], in0=cd0[:, :],
                                            in1=tmp[:, :], op=ALU.add)
                    nc.vector.tensor_scalar(out=tmp[:, :], in0=P3, scalar1=a3[:, 0:1],
                                            scalar2=None, op0=ALU.mult)
                    nc.vector.tensor_tensor(out=cd0[:, :], in0=cd0[:, :],
                                            in1=tmp[:, :], op=ALU.add)
                    # cdp top = b*P1 + 2ab*P2 + 3a^2b*P3  (x coef)
                    nc.vector.tensor_scalar(out=cdpf[:, :], in0=P1, scalar1=bb[:, 0:1],
                                            scalar2=None, op0=ALU.mult)
                    nc.vector.tensor_scalar(out=tmp[:, :], in0=P2, scalar1=ab[:, 0:1],
                                            scalar2=2.0, op0=ALU.mult, op1=ALU.mult)
                    nc.vector.tensor_tensor(out=cdpf[:, :], in0=cdpf[:, :],
                                            in1=tmp[:, :], op=ALU.add)
                    nc.vector.tensor_scalar(out=tmp[:, :], in0=P3, scalar1=a2b[:, 0:1],
                                            scalar2=3.0, op0=ALU.mult, op1=ALU.mult)
                    nc.vector.tensor_tensor(out=cdpf[:, :], in0=cdpf[:, :],
                                            in1=tmp[:, :], op=ALU.add)
                    # cdp bottom = b^3*P3  (x^3 coef)
                    nc.vector.tensor_scalar(out=cdpf[64:128, :], in0=P3[64:128, :],
                                            scalar1=b3[64:128, 0:1],
                                            scalar2=None, op0=ALU.mult)
                    # cdq (64 rows) = b^2*P2 + 3ab^2*P3  (x^2 coef)
                    nc.vector.tensor_scalar(out=cdqf[:, :], in0=P2[0:64, :],
                                            scalar1=b2[0:64, 0:1],
                                            scalar2=None, op0=ALU.mult)
                    nc.vector.tensor_scalar(out=tmp[0:64, :], in0=P3[0:64, :],
                                            scalar1=ab2[0:64, 0:1],
                                            scalar2=3.0, op0=ALU.mult, op1=ALU.mult)
                    nc.vector.tensor_tensor(out=cdqf[:, :], in0=cdqf[:, :],
                                            in1=tmp[0:64, :], op=ALU.add)
                    cwpd = pers.tile([128, OUT], hf, tag="cwpd")
                    nc.vector.tensor_tensor(out=cwpd[:, :], in0=cwf[:, 0:32],
                                            in1=cdpf[:, :], op=ALU.add)
                    cwqd = pers.tile([128, OUT], hf, tag="cwqd")
                    nc.vector.tensor_copy(out=cwqd[64:128, :],
                                          in_=cwf[64:128, 32:64])
                    nc.vector.tensor_tensor(out=cwqd[0:64, :],
                                            in0=cwf[0:64, 32:64],
                                            in1=cdqf[:, :], op=ALU.add)

                    # rho chunk coefs (fp16): cols 0..3 top/bottom = RW[j]*b^3;
                    # col3 bottom = bias rows (cot + cd0 sum plane)
                    rhw = pers.tile([128, 4 * OUT], hf, tag="rhw")
                    for j in range(1, 8):
                        q, half = (j - 1) // 2, (j - 1) % 2
                        r0, r1 = half * 64, (half + 1) * 64
                        nc.vector.tensor_scalar(
                            out=rhw[r0:r1, 32 * q:32 * (q + 1)],
                            in0=rwt[r0:r1, 32 * (j - 1):32 * j],
                            scalar1=b3[r0:r1, 0:1], scalar2=None, op0=ALU.mult)
                    cot2 = pers.tile([IN, OUT], dt, tag="cot2")
                    nc.vector.tensor_tensor(out=cot2[:, :], in0=cot[:, :],
                                            in1=cd0[0:64, :], op=ALU.add)
                    nc.vector.tensor_copy(out=rhw[64:128, 96:128], in_=cot2[:, :])
                    return kp, rhw, cwpd, cwqd

                # =========== phase D per super (rho + dyn chunks + out) ====
                def phase_d(s, kp, rhw, cwpd, cwqd):
                    xsl = xds[s][:, :]
                    acc = accs.pop(s)
                    for q in range([] if skip_rho else 4) if False else range(0 if skip_rho else 4):
                        if q < 3:
                            rq = plane.tile([128, SBC], hf, tag="pl")
                            nc.vector._custom_dve(OPS["ANT_FK_RELUCUBE"],
                                                  out=rq[:, :], in0=xsl,
                                                  s0=kp[:, q:q + 1])
                        else:
                            rq = rq3s[s]
                            nc.vector._custom_dve(OPS["ANT_FK_RELUCUBE"],
                                                  out=rq[0:64, :],
                                                  in0=xsl[0:64, :],
                                                  s0=kp[0:64, q:q + 1])
                        nc.tensor.matmul(acc[:, 0:BC], rhw[:, 32 * q:32 * (q + 1)],
                                         rq[:, 0:BC], start=False, stop=False)
                        nc.tensor.matmul(acc[:, BC:SBC], rhw[:, 32 * q:32 * (q + 1)],
                                         rq[:, BC:SBC], start=False, stop=False)
                    Pp = Ps.pop(s)
                    nc.tensor.matmul(acc[:, 0:BC], cwpd[:, :], Pp[:, 0:BC],
                                     start=False, stop=False)
                    nc.tensor.matmul(acc[:, BC:SBC], cwpd[:, :], Pp[:, BC:SBC],
                                     start=False, stop=False)
                    Qp = Qs.pop(s)
                    nc.tensor.matmul(acc[:, 0:BC], cwqd[:, :], Qp[:, 0:BC],
                                     start=False, stop=True)
                    nc.tensor.matmul(acc[:, BC:SBC], cwqd[:, :], Qp[:, BC:SBC],
                                     start=False, stop=True)
                    # undo the phase-A batch permutation (col t*128+p ->
                    # batch 8p+t) inside the PSUM->SBUF copy's read AP
                    yt = ytp.tile([OUT, SBC], dt, tag="yt")
                    nc.scalar.copy(
                        yt.rearrange("o (p g t) -> o p g t", p=128, g=2, t=4),
                        acc.rearrange("o (g t p) -> o p g t", g=2, t=4,
                                      p=128))
                    nc.scalar.dma_start(out=y_d[:, s * SBC:(s + 1) * SBC],
                                        in_=yt[:, :])

                if phases == "A0":
                    ydum = ytp.tile([OUT, SBC], dt, tag="yt")
                    nc.vector.memset(ydum[:, :], 0.0)
                    nc.vector.tensor_copy(out=ydum[:, 0:NSUP],
                                          in_=mm[0:OUT, :])
                    for s in range(NSUP):
                        nc.sync.dma_start(out=y_d[:, s * SBC:(s + 1) * SBC],
                                          in_=ydum[:, :])
                    if _rep + 1 < reps:
                        phase_a_all()
                    continue
                gm2 = phase_coll()
                if phases == "A":
                    ydum = ytp.tile([OUT, SBC], dt, tag="yt")
                    nc.vector.memset(ydum[:, :], 0.0)
                    nc.vector.tensor_copy(out=ydum[:, 0:16], in_=gm2[0:OUT, :])
                    for s in range(NSUP):
                        nc.sync.dma_start(out=y_d[:, s * SBC:(s + 1) * SBC],
                                          in_=ydum[:, :])
                    if _rep + 1 < reps:
                        phase_a_all()
                    continue
                phase_b(0)
                phase_b(1)
                phase_b(2)
                phase_b(3, defer=True)
                phase_b_wav(0)
                phase_b_wav(1)
                phase_b_wav(2)
                phase_b_wav(3, defer=True)
                if _rep + 1 < reps:
                    phase_a_all()
                dyn = phase_c(gm2)
                phase_d(0, *dyn)
                phase_b_mm(3)
                phase_b_wav_mm(3)
                phase_d(1, *dyn)
                phase_d(2, *dyn)
                phase_d(3, *dyn)
    nc.compile()
    return nc


_NC_CACHE = None


def _get_nc():
    global _NC_CACHE
    if _NC_CACHE is None:
        _NC_CACHE = build_nc()
    return _NC_CACHE


def make_in_maps(inputs):
    consts = fold_constants(inputs)
    x = np.asarray(inputs["x"], np.float16)
    xdup = np.ascontiguousarray(np.concatenate([x, x], axis=1))
    in_maps = []
    for c in range(N_CORES):
        m = {"xs": xdup[c * BS:(c + 1) * BS]}
        m.update(consts)
        in_maps.append(m)
    return in_maps


def kernel(**inputs) -> np.ndarray:
    from concourse.bass_utils import run_bass_kernel_spmd
    nc = _get_nc()
    in_maps = make_in_maps(inputs)
    res = run_bass_kernel_spmd(nc, in_maps, core_ids=list(range(N_CORES)))
    out = np.concatenate([res.results[c]["y"].T for c in range(N_CORES)], axis=0)
    return np.ascontiguousarray(out, dtype=F32)


# revision 35
# speedup vs baseline: 1.1980x; 1.1980x over previous
"""FKANLinear fused kernel v3 for 8 TRN2 NeuronCores.

Changes vs v2 baseline (engine rebalance; DVE was 72% busy, span 102us):
- fp16 planes/coefs everywhere (same matmul & DVE speed as bf16, 8x the
  mantissa).
- Fourier seeds without custom ISA: c2m = Sin(x/2 + [0;pi/2]) gives
  [sin(x/2); cos(x/2)] in one ACT op; q = s2*c2m; p1 = ts(q) -> [cos x;
  sin x]; cc = Square(c2m) on ACT; cdup = ts(cc) -> [cos x; cos x].
  ts (TensorScalar) runs in DVE 4x mode = 327ns/plane.
- Monomial chunks re-paired as P=[x;x^3] (1 ISA) and Q=TT(P,xd)=[x^2;x^4]
  (1 TT) instead of two ISA ops.
- Wavelet (u^2-1)e^{-u^2/2} split into two PE chunks: TT(u2,ew) and the
  ew plane itself with negated coefficients (PE has headroom; saves DVE).
- Single min/max reduce per super over a ts-built [-x; x] plane.
- Bias chunk folded into the spare bottom half of the 4th rho plane.
- Fourier chain TTs split DVE/Pool (tail of chain on Pool).
- y written by DMA straight from PSUM; xd copies via ACT (Copy is in
  every act table).
"""

import sys
import numpy as np

if "/opt/trn_rl_repo" not in sys.path:
    sys.path.insert(0, "/opt/trn_rl_repo")

# ---------------------------------------------------------------------------
# custom DVE op registration (runtime; table ships in the NEFF)
# ---------------------------------------------------------------------------

_REGISTERED = {}


def register_ops():
    global _REGISTERED
    if _REGISTERED:
        return _REGISTERED
    from concourse import dve_ops
    from concourse.dve_spec import (Spec, Src0, Src1, C0, C1, One, sq, relu,
                                    select, lower)
    from concourse.dve_spec import _has_src1 as has_src1
    from concourse.dve_uop import DveOpSpec

    s_ = sq(Src0)
    r_ = relu(Src0 + C0)
    defs = {
        # [x ; x^3] from xdup: s0 = 0 top / 1 bottom
        "ANT_FK_M13": Spec(
            body=select(C0, s_ * Src0, Src0),
            reference=lambda in0, in1, s0, s1, imm2: np.where(
                np.asarray(s0) != 0, in0 ** 3, in0),
        ),
        # relu(x + kappa)^3 (kappa = s0 per partition)
        "ANT_FK_RELUCUBE": Spec(
            body=sq(r_) * r_,
            reference=lambda in0, in1, s0, s1, imm2: np.maximum(
                in0 + np.asarray(s0), 0.0) ** 3,
        ),
    }

    base = max(dve_ops._SUB_OPCODE_FOR_NAME.values()) + 1
    for i, (name, spec) in enumerate(defs.items()):
        if name in dve_ops._SUB_OPCODE_FOR_NAME:
            continue
        opcode = base + i
        assert opcode < 0x20, "DVE opcode rows exhausted"
        dve_ops._SUB_OPCODE_FOR_NAME[name] = opcode
        shas = {}
        for ver in ("v3", "v4"):
            uops = lower(spec, ver=ver)
            shas[ver] = DveOpSpec(name=name, opcode=opcode, uops=uops,
                                  rd1_en=has_src1(spec)).sha(ver)
        op = dve_ops.DveOp(name, spec, subdim=False, uops_sha=shas)
        dve_ops.OPS.append(op)
        dve_ops.CUSTOM_DVE_SPECS[name] = spec
        _REGISTERED[name] = op
    if not _REGISTERED:
        by_name = {op.name: op for op in dve_ops.OPS}
        _REGISTERED = {n: by_name[n] for n in defs}
    return _REGISTERED


N_CORES = 8
B, IN, OUT = 32768, 64, 32
BS = B // N_CORES          # 4096 rows per core
SBC = 1024                 # batch columns per super-block
NSUP = BS // SBC           # 4
BC = 512                   # matmul moving-dim chunk
GRP = 512                  # rows per x-load DMA
G, P = 8, 3
TAY = 4
JDEG, JA, JB = 4, 1.0, 1.0
CDEG = 4
FREQ = 8
WCH = 4
TEMP = 2.0

F32 = np.float32

# ----------------------------------------------------------------------------
# host-side folding
# ----------------------------------------------------------------------------

def _softplus(z):
    z = np.asarray(z, np.float64)
    return np.log1p(np.exp(-np.abs(z))) + np.maximum(z, 0.0)


def _softmax(z, axis):
    z = np.asarray(z, np.float64)
    m = z.max(axis=axis, keepdims=True)
    e = np.exp(z - m)
    return e / e.sum(axis=axis, keepdims=True)


def _jacobi_mono():
    a, b = JA, JB
    terms = np.zeros((JDEG + 1, 5))
    terms[0, 0] = 1.0
    if JDEG >= 1:
        terms[1, 1] = 0.5 * 2.0 * (a + 1.0) / np.sqrt(2.0)
        terms[1, 0] = 0.5 * (a - b) / np.sqrt(2.0)
    for n in range(2, JDEG + 1):
        k = n - 1
        A1 = 2 * k + a + b
        A2 = 2 * (k + 1) * (k + a + b + 1) * (A1 + 1)
        A4 = 2 * (k + a) * (k + b) * (A1 + 2)
        c_x = (A1 + 1) * (A1 + 2) * A1 / A2
        c_0 = (A1 + 1) * (a * a - b * b) / A2
        Jn = np.zeros(5)
        Jn[1:] += c_x * terms[n - 1][:4]
        Jn += c_0 * terms[n - 1]
        Jn -= (A4 / A2) * terms[n - 2]
        terms[n] = Jn / np.sqrt(n + 1.0)
    return terms


def _cheby_mono():
    T = np.zeros((CDEG + 1, 5))
    T[0, 0] = 1.0
    T[1, 1] = 1.0
    for n in range(2, CDEG + 1):
        shift = np.zeros(5)
        shift[1:] = T[n - 1][:4]
        T[n] = 2.0 * shift - T[n - 2]
    norm = 1.0 / np.sqrt(np.arange(CDEG + 1) + 1.0)
    return T * norm[:, None]


def _bspline_tspace_phi(t):
    grid = np.concatenate([np.zeros(3), np.linspace(0.0, 8.0, G + 1), np.full(3, 8.0)])
    te = t[:, None]
    bases = ((te >= grid[None, :-1]) & (te < grid[None, 1:])).astype(np.float64)
    mask_last = t == grid[-1]
    bases[mask_last, :] = 0.0
    bases[mask_last, -1] = 1.0
    for r in range(1, P + 1):
        ld = np.maximum(grid[r:-1] - grid[:-(r + 1)], 1e-12)
        rd = np.maximum(grid[r + 1:] - grid[1:-r], 1e-12)
        left = (te - grid[None, :-(r + 1)]) / ld[None, :] * bases[:, :-1]
        right = (grid[None, r + 1:] - te) / rd[None, :] * bases[:, 1:]
        bases = left + right
    return bases


def _bspline_truncpow_matrix():
    S = 6000
    t = np.linspace(0.0, 8.0, S)
    t = t + 1e-7
    t = np.clip(t, 0.0, 8.0)
    phi = _bspline_tspace_phi(t)
    Fm = np.zeros((S, 11))
    Fm[:, 0] = 1.0
    Fm[:, 1] = t
    Fm[:, 2] = t * t
    Fm[:, 3] = t ** 3
    for j in range(1, 8):
        Fm[:, 3 + j] = np.maximum(t - j, 0.0) ** 3
    M, _, _, _ = np.linalg.lstsq(Fm, phi, rcond=None)
    return M


NCH = 14   # static chunk coef columns: P,Q, f1..f8, wA0,wA1, wB0,wB1


def fold_constants(inputs):
    base_v = np.asarray(inputs["base_v"], np.float64)
    base_g = np.asarray(inputs["base_g"], np.float64)
    base_bias = np.asarray(inputs["base_bias"], np.float64)
    gains = np.asarray(inputs["gains"], np.float64)
    alpha = float(_softplus(inputs["alpha_logit"]))
    beta = float(_softplus(inputs["beta_logit"]))
    mixw = _softmax(np.asarray(inputs["mix_logits"], np.float64) / TEMP, axis=-1)
    sg = _softplus(gains)

    def ceff(name, f):
        return np.asarray(inputs[name], np.float64) * mixw[..., f:f + 1] * sg[f] * beta

    C_bs = ceff("bspline_coef", 0)
    C_ty = ceff("taylor_coef", 1)
    C_jb = ceff("jacobi_coef", 2)
    C_cb = ceff("cheby_coef", 3)
    C_fr = ceff("fourier_coef", 4)
    C_wv = ceff("wavelet_coef", 5)

    vn = np.sqrt((base_v ** 2).sum(axis=1, keepdims=True))
    Walpha = alpha * base_g * base_v / vn
    bias_alpha = alpha * base_bias

    mono = np.zeros((OUT, IN, 5))
    fac = np.array([1.0, 1.0, 2.0, 6.0])
    mono[:, :, :4] += C_ty / fac[None, None, :]
    mono += np.einsum("oin,nd->oid", C_jb, _jacobi_mono())
    mono += np.einsum("oin,nd->oid", C_cb, _cheby_mono())

    fnorm = 1.0 / np.sqrt(2.0 * FREQ)
    Ccos = C_fr[:, :, :FREQ] * fnorm
    Csin = C_fr[:, :, FREQ:] * fnorm
    Tc = np.zeros((9, 9)); Tc[0, 0] = 1.0; Tc[1, 1] = 1.0
    Uc = np.zeros((9, 9)); Uc[0, 0] = 1.0; Uc[1, 1] = 2.0
    for n in range(2, 9):
        for M_ in (Tc, Uc):
            sh = np.zeros(9); sh[1:] = M_[n - 1][:8]
            M_[n] = 2.0 * sh - M_[n - 2]
    Ccpow = np.einsum("oik,kj->oij", Ccos, Tc[1:9, :])
    Cspow = np.einsum("oik,kj->oij", Csin, Uc[0:8, :])

    a_w = _softplus(np.asarray(inputs["wavelet_scale_logit"], np.float64)) + 1e-6
    inva = 1.0 / a_w
    nshia = -np.asarray(inputs["wavelet_shift"], np.float64) * inva

    M = _bspline_truncpow_matrix()
    CF = np.einsum("oik,mk->oim", C_bs, M)
    P_poly = CF[:, :, :4]
    R_rho = CF[:, :, 4:]

    # static chunk coefs: (128, NCH*32), fp16 on device
    CW = np.zeros((128, NCH * OUT))
    def put(c, half, arr_oi):
        CW[half * 64:(half + 1) * 64, 32 * c:32 * (c + 1)] = arr_oi.T
    # c0: P=[x; x^3], c1: Q=[x^2; x^4]
    put(0, 0, Walpha + mono[:, :, 1])
    put(0, 1, mono[:, :, 3])
    put(1, 0, mono[:, :, 2])
    put(1, 1, mono[:, :, 4])
    # c2..c9: fourier powers [cos^j; sin*cos^(j-1)]
    for j in range(1, 9):
        put(1 + j, 0, Ccpow[:, :, j])
        put(1 + j, 1, Cspow[:, :, j - 1])
    # c10,c11: wavelet A chunks (u2*ew); c12,c13: ew chunks (coef = -A)
    put(10, 0, C_wv[:, :, 0]); put(10, 1, C_wv[:, :, 1])
    put(11, 0, C_wv[:, :, 2]); put(11, 1, C_wv[:, :, 3])
    put(12, 0, -C_wv[:, :, 0]); put(12, 1, -C_wv[:, :, 1])
    put(13, 0, -C_wv[:, :, 2]); put(13, 1, -C_wv[:, :, 3])

    CO = (mono[:, :, 0] + Ccpow[:, :, 0]).T.copy()
    CO[0, :] += bias_alpha

    PW = np.zeros((128, 4 * OUT))
    for d in range(4):
        PW[0:64, 32 * d:32 * (d + 1)] = P_poly[:, :, d].T
    PW[64:128] = PW[0:64]
    RW = np.zeros((128, 7 * OUT))
    for j in range(7):
        RW[0:64, 32 * j:32 * (j + 1)] = R_rho[:, :, j].T
    RW[64:128] = RW[0:64]

    WVP = np.zeros((128, 4))
    for p in range(2):
        WVP[0:64, 2 * p] = inva[:, 2 * p]
        WVP[64:128, 2 * p] = inva[:, 2 * p + 1]
        WVP[0:64, 2 * p + 1] = nshia[:, 2 * p]
        WVP[64:128, 2 * p + 1] = nshia[:, 2 * p + 1]

    ID = np.eye(128)

    return {
        "CW": CW.astype(F32), "CO": CO.astype(F32), "PW": PW.astype(F32),
        "RW": RW.astype(F32), "WVP": WVP.astype(F32), "ID": ID.astype(F32),
    }


# ----------------------------------------------------------------------------
# numpy emulation (validates folding; mirrors device chunk structure)
# ----------------------------------------------------------------------------

def numpy_forward(inputs):
    consts = fold_constants(inputs)
    x = np.asarray(inputs["x"], np.float64)
    CW = consts["CW"].astype(np.float64)
    CO = consts["CO"].astype(np.float64)
    PW = consts["PW"].astype(np.float64)
    RW = consts["RW"].astype(np.float64)
    WVP = consts["WVP"].astype(np.float64)

    xmin = x.min(axis=0); xmax = x.max(axis=0)
    pad = (xmax - xmin) < 1e-8
    xmin = np.where(pad, xmin - 0.5, xmin)
    xmax = np.where(pad, xmax + 0.5, xmax)
    rng = xmax - xmin
    b = 8.0 / rng
    a = -xmin * b
    P_poly = np.stack([PW[0:64, 32 * d:32 * (d + 1)] for d in range(4)], axis=-1)
    binom = {(0, 0): 1, (1, 0): 1, (1, 1): 1, (2, 0): 1, (2, 1): 2, (2, 2): 1,
             (3, 0): 1, (3, 1): 3, (3, 2): 3, (3, 3): 1}
    Cdyn = np.zeros((IN, OUT, 4))
    for d in range(4):
        for e in range(d + 1):
            Cdyn[:, :, e] += P_poly[:, :, d] * (binom[(d, e)] * a ** (d - e) * b ** e)[:, None]
    # dyn-mono chunk coefs for P=[x;x^3] and Q=[x^2;-] planes
    CDP = np.zeros((128, OUT)); CDQ = np.zeros((64, OUT))
    CDP[0:64] = Cdyn[:, :, 1]; CDP[64:128] = Cdyn[:, :, 3]
    CDQ[0:64] = Cdyn[:, :, 2]
    bias = (CO + Cdyn[:, :, 0]).sum(axis=0)
    # rho chunk coefs; col 3 bottom carries the bias row-block (vs ones plane)
    RHW = np.zeros((128, 4 * OUT))
    for j in range(1, 8):
        q, half = (j - 1) // 2, (j - 1) % 2
        RHW[half * 64:(half + 1) * 64, 32 * q:32 * (q + 1)] = \
            RW[0:64, 32 * (j - 1):32 * j] * (b ** 3)[:, None]
    RHW[64:128, 96:128] = (CO + Cdyn[:, :, 0])

    Bn = x.shape[0]
    y = np.zeros((Bn, OUT))
    kap = [xmin + j * rng / 8.0 for j in range(1, 8)]

    feats = []
    # P, Q monomials
    f = np.zeros((Bn, 128)); f[:, 0:64] = x; f[:, 64:128] = x ** 3
    feats.append((f, CW[:, 0:32], 128))
    f = np.zeros((Bn, 128)); f[:, 0:64] = x * x; f[:, 64:128] = x ** 4
    feats.append((f, CW[:, 32:64], 128))
    # fourier powers
    cc_, ss_ = np.cos(x), np.sin(x)
    for j in range(1, 9):
        f = np.zeros((Bn, 128))
        f[:, 0:64] = cc_ ** j; f[:, 64:128] = ss_ * cc_ ** (j - 1)
        feats.append((f, CW[:, 32 * (1 + j):32 * (2 + j)], 128))
    # wavelets: A = u2*ew, B = ew
    for p in range(2):
        u0 = x * WVP[None, 0:64, 2 * p] + WVP[None, 0:64, 2 * p + 1]
        u1 = x * WVP[None, 64:128, 2 * p] + WVP[None, 64:128, 2 * p + 1]
        e0 = np.exp(-0.5 * u0 ** 2); e1 = np.exp(-0.5 * u1 ** 2)
        f = np.zeros((Bn, 128))
        f[:, 0:64] = u0 ** 2 * e0; f[:, 64:128] = u1 ** 2 * e1
        feats.append((f, CW[:, 32 * (10 + p):32 * (11 + p)], 128))
        f = np.zeros((Bn, 128))
        f[:, 0:64] = e0; f[:, 64:128] = e1
        feats.append((f, CW[:, 32 * (12 + p):32 * (13 + p)], 128))
    # rho (col 3: knot 7 top, ones bottom for bias)
    for q in range(4):
        j0 = 2 * q + 1
        f = np.zeros((Bn, 128))
        f[:, 0:64] = np.maximum(x - kap[j0 - 1][None, :], 0.0) ** 3
        if j0 + 1 <= 7:
            f[:, 64:128] = np.maximum(x - kap[j0][None, :], 0.0) ** 3
        else:
            f[:, 64:128] = 1.0
        feats.append((f, RHW[:, 32 * q:32 * (q + 1)], 128))
    # dyn-mono
    f = np.zeros((Bn, 128)); f[:, 0:64] = x; f[:, 64:128] = x ** 3
    feats.append((f, CDP, 128))
    f = np.zeros((Bn, 128)); f[:, 0:64] = x * x
    feats.append((f, CDQ, 64))

    for f, w, rows in feats:
        y = y + f[:, :rows] @ w[:rows]
    return y.astype(F32)


# ----------------------------------------------------------------------------
# device kernel
# ----------------------------------------------------------------------------

def build_nc(reps=1, no_collective=False, phases='full', n_pool_f=0,
             y_dma=True, plane_fp16=True, pool_q=False, skip_rho=False,
             nf=8, skip_wav=False):
    import concourse.bass as bass
    import concourse.bacc as bacc
    import concourse.mybir as mybir
    import concourse.tile as tile

    OPS = register_ops()

    dt = mybir.dt.float32
    hf = mybir.dt.float16 if plane_fp16 else mybir.dt.bfloat16
    AF = mybir.ActivationFunctionType
    ALU = mybir.AluOpType
    AX = mybir.AxisListType

    nc = bacc.Bacc("TRN2", target_bir_lowering=False, debug=False,
                   enable_asserts=True, num_devices=N_CORES)

    xs = nc.dram_tensor("xs", [128, NSUP * 8 * 2 * IN], mybir.dt.float16,
                        kind="ExternalInput").ap()
    cw_d = nc.dram_tensor("CW", [128, NCH * OUT], dt, kind="ExternalInput").ap()
    co_d = nc.dram_tensor("CO", [IN, OUT], dt, kind="ExternalInput").ap()
    pw_d = nc.dram_tensor("PW", [128, 4 * OUT], dt, kind="ExternalInput").ap()
    rw_d = nc.dram_tensor("RW", [128, 7 * OUT], dt, kind="ExternalInput").ap()
    wv_d = nc.dram_tensor("WVP", [128, 4], dt, kind="ExternalInput").ap()
    id_d = nc.dram_tensor("ID", [128, 128], dt, kind="ExternalInput").ap()
    y_d = nc.dram_tensor("y", [OUT, BS], dt, kind="ExternalOutput").ap()

    def cw(c):
        return cwt[:, 32 * c:32 * (c + 1)]

    with tile.TileContext(nc) as tc:
        with (
            tc.tile_pool(name="const", bufs=1) as cpool,
            tc.tile_pool(name="plane", bufs=8) as plane,
            tc.tile_pool(name="seed", bufs=4) as seedp,
            tc.tile_pool(name="cdw", bufs=4) as cdwp,
            tc.tile_pool(name="ytp", bufs=2) as ytp,
            tc.tile_pool(name="xpipe", bufs=3) as xpipe,
            tc.tile_pool(name="pers", bufs=1) as pers,
            tc.tile_pool(name="xdp", bufs=2) as xdp,
            tc.tile_pool(name="ps", bufs=2, space="PSUM") as ps,
            tc.tile_pool(name="psacc", bufs=3, space="PSUM") as psacc,
            tc.tile_pool(name="dram", bufs=1, space="DRAM") as dram,
        ):
            # ---- constants ----
            cwf = cpool.tile([128, NCH * OUT], dt, tag="cwf")
            cot = cpool.tile([IN, OUT], dt, tag="cot")
            pwt = cpool.tile([128, 4 * OUT], dt, tag="pwt")
            rwt = cpool.tile([128, 7 * OUT], dt, tag="rwt")
            wvt = cpool.tile([128, 4], dt, tag="wvt")
            idt = cpool.tile([128, 128], dt, tag="idt")
            idt16 = cpool.tile([128, 128], mybir.dt.float16, tag="idt16")
            nc.sync.dma_start(out=cwf[:, :], in_=cw_d[:, :])
            nc.sync.dma_start(out=cot[:, :], in_=co_d[:, :])
            nc.sync.dma_start(out=pwt[:, :], in_=pw_d[:, :])
            nc.sync.dma_start(out=rwt[:, :], in_=rw_d[:, :])
            nc.sync.dma_start(out=wvt[:, :], in_=wv_d[:, :])
            nc.sync.dma_start(out=idt[:, :], in_=id_d[:, :])
            cwt = cpool.tile([128, NCH * OUT], hf, tag="cwt")
            nc.vector.tensor_copy(out=cwt[:, :], in_=cwf[:, :])
            nc.vector.tensor_copy(out=idt16[:, :], in_=idt[:, :])

            # per-partition constant columns
            selneg = cpool.tile([128, 1], dt, tag="selneg")   # [-1;+1]
            nc.vector.memset(selneg[0:64, :], -1.0)
            nc.vector.memset(selneg[64:128, :], 1.0)
            selpi2 = cpool.tile([128, 1], dt, tag="selpi2")   # [0; pi/2]
            nc.vector.memset(selpi2[0:64, :], 0.0)
            nc.vector.memset(selpi2[64:128, :], float(np.pi / 2))
            selm2p2 = cpool.tile([128, 1], dt, tag="selm2p2")  # [-2; +2]
            nc.vector.memset(selm2p2[0:64, :], -2.0)
            nc.vector.memset(selm2p2[64:128, :], 2.0)
            sel10 = cpool.tile([128, 1], dt, tag="sel10")      # [1; 0]
            nc.vector.memset(sel10[0:64, :], 1.0)
            nc.vector.memset(sel10[64:128, :], 0.0)
            sel1m1 = cpool.tile([128, 1], dt, tag="sel1m1")    # [1; -1]
            nc.vector.memset(sel1m1[0:64, :], 1.0)
            nc.vector.memset(sel1m1[64:128, :], -1.0)
            sel01 = cpool.tile([128, 1], dt, tag="sel01")      # [0; 1]
            nc.vector.memset(sel01[0:64, :], 0.0)
            nc.vector.memset(sel01[64:128, :], 1.0)
            selhalf = cpool.tile([128, 1], dt, tag="selhalf")  # 0.5 everywhere
            nc.vector.memset(selhalf[:, :], 0.5)

            # rho plane q=3: bottom half is a persistent all-ones block
            # (bias chunk rides on it); top half rewritten per rep/super.
            rq3s = []
            for s in range(NSUP):
                t_ = cpool.tile([128, SBC], hf, tag=f"rq3_{s}")
                nc.vector.memset(t_[64:128, :], 1.0)
                rq3s.append(t_)

            state = {}

            def phase_a_all():
                # =========== phase A (all supers of one rep) ==============
                mm = xdp.tile([128, NSUP], dt, tag="mm")
                xds = [None] * NSUP
                state["mm"] = mm
                state["xds"] = xds

                # one x DMA per rep: host lays out each partition's whole
                # rep data contiguously (xs[p, (s,t,i)] = x[s*1024+8p+t, i]).
                # xd column g*512+t*128+p holds batch row s*1024+8p+4g+t;
                # the y copy undoes the permutation.
                xt8 = xpipe.tile([128, NSUP * 16 * IN], mybir.dt.float16,
                                 tag="xin", bufs=2)
                nc.sync.dma_start(out=xt8[:, :], in_=xs[:, :])

                def phase_a(s):
                    xd = xdp.tile([128, SBC], hf, tag=f"xd{s}")
                    xds[s] = xd
                    xsup = xt8[:, s * 16 * IN:(s + 1) * 16 * IN]
                    for g in range(2):
                        tp = ps.tile([128, 512], mybir.dt.float16,
                                     tag="tp")
                        for t in range(4):
                            tt = 4 * g + t
                            nc.tensor.transpose(
                                tp[:, 128 * t:128 * (t + 1)],
                                xsup[:, tt * 128:(tt + 1) * 128], idt16[:, :])
                        nc.scalar.copy(xd[:, g * 512:(g + 1) * 512], tp[:, :])
                    # [-x; x] then one max-reduce -> [-min; max] per column
                    ng = seedp.tile([128, SBC], hf, tag="ng")
                    nc.vector.tensor_scalar(out=ng[:, :], in0=xd[:, :],
                                            scalar1=selneg[:, 0:1],
                                            scalar2=None, op0=ALU.mult)
                    nc.vector.tensor_reduce(out=mm[:, s:s + 1], in_=ng[:, :],
                                            axis=AX.X, op=ALU.max)

                for s in range(NSUP):
                    phase_a(s)

            phase_a_all()
            for _rep in range(reps):
                mm = state["mm"]
                xds = state["xds"]
                locmm = xdp.tile([128, 1], dt, tag="locmm")
                def phase_coll():
                    nc.vector.tensor_reduce(out=locmm[:, 0:1], in_=mm[:, 0:NSUP],
                                            axis=AX.X, op=ALU.max)
                    return _coll_body()

                # ====== collective: all-reduce-max per-column [-min;max] ==
                def _coll_body():
                    bounce_in = dram.tile([2, IN], dt, tag="cin")
                    bounce_out = dram.tile([2, IN], dt, tag="cout")
                    nc.sync.dma_start(
                        out=bounce_in.rearrange("a b -> (a b)"),
                        in_=locmm[:, 0:1].rearrange("p x -> (p x)"))
                    if no_collective:
                        nc.sync.dma_start(out=bounce_out[:, :],
                                          in_=bounce_in[:, :])
                    else:
                        nc.gpsimd.collective_compute(
                            "AllReduce", mybir.AluOpType.max,
                            replica_groups=[list(range(N_CORES))],
                            ins=[bounce_in.opt()],
                            outs=[bounce_out.opt()],
                        )
                    # gm[:,0] = -gmin, gm[:,1] = gmax on every partition
                    gm = xdp.tile([128, 2], dt, tag="gm")
                    nc.sync.dma_start(out=gm[0:IN, :],
                                      in_=bounce_out.rearrange("a b -> b a"))
                    nc.sync.dma_start(out=gm[IN:128, :],
                                      in_=bounce_out.rearrange("a b -> b a"))
                    return gm

                # =========== phase B per super (chunks 0..9) ==============
                accs = {}
                Ps = {}
                Qs = {}
                bdefer = {}
                wdefer = {}

                def mmc_on(acc, lhs, F, first=False, rows=128,
                           stop=False):
                    nc.tensor.matmul(acc[:, 0:BC], lhs[0:rows, :],
                                     F[0:rows, 0:BC], start=first, stop=stop)
                    nc.tensor.matmul(acc[:, BC:SBC], lhs[0:rows, :],
                                     F[0:rows, BC:SBC], start=first, stop=stop)

                def phase_b(s, defer=False):
                    xsl = xds[s][:, :]
                    if defer:
                        chunks = bdefer.setdefault(s, [])
                        emit = lambda c, F: chunks.append((c, F))
                    else:
                        acc = psacc.tile([OUT, SBC], dt, tag="acc")
                        accs[s] = acc
                        emit = lambda c, F: mmc_on(acc, cw(c), F, first=(c == 2))
                    ptag = "pl3" if defer else "pl"
                    pbufs = 8

                    # monomial planes P=[x;x^3] (ISA), Q=[x^2;x^4]=P*xd (TT)
                    Pp = xdp.tile([128, SBC], hf, tag=f"P_{s}")
                    Ps[s] = Pp
                    nc.vector._custom_dve(OPS["ANT_FK_M13"], out=Pp[:, :],
                                          in0=xsl, s0=sel01[:, 0:1])
                    Qp = xdp.tile([128, SBC], hf, tag=f"Q_{s}")
                    Qs[s] = Qp
                    qeng = nc.gpsimd if pool_q else nc.vector
                    qeng.tensor_tensor(out=Qp[:, :], in0=Pp[:, :],
                                       in1=xsl, op=ALU.mult)

                    # fourier seeds: s2=[sin(x/2)]², c2m=[sin(x/2); cos(x/2)]
                    s2 = seedp.tile([128, SBC], hf, tag="s2")
                    nc.scalar.activation(s2[:, :], xsl, AF.Sin, scale=0.5)
                    c2m = seedp.tile([128, SBC], hf, tag="c2m")
                    nc.scalar.activation(c2m[:, :], xsl, AF.Sin, scale=0.5,
                                         bias=selpi2[:, 0:1])
                    # cc = Square(c2m) = [sin²(x/2); cos²(x/2)]  (ACT)
                    ccp = cdwp.tile([128, SBC], hf, tag="ccp")
                    nc.scalar.activation(ccp[:, :], c2m[:, :], AF.Square)
                    # q = [sin²(x/2); sin(x/2)cos(x/2)]
                    qt = seedp.tile([128, SBC], hf, tag="qt")
                    nc.vector.tensor_tensor(out=qt[:, :], in0=s2[:, :],
                                            in1=c2m[:, :], op=ALU.mult)
                    # p1 = q*[-2;2] + [1;0] = [cos x; sin x]
                    pk = plane.tile([128, SBC], hf, tag=ptag, bufs=pbufs)
                    nc.vector.tensor_scalar(out=pk[:, :], in0=qt[:, :],
                                            scalar1=selm2p2[:, 0:1],
                                            scalar2=sel10[:, 0:1],
                                            op0=ALU.mult, op1=ALU.add)
                    # cdup = cc*[-2;2] + [1;-1] = [cos x; cos x]
                    cdup = cdwp.tile([128, SBC], hf, tag="cdup")
                    nc.vector.tensor_scalar(out=cdup[:, :], in0=ccp[:, :],
                                            scalar1=selm2p2[:, 0:1],
                                            scalar2=sel1m1[:, 0:1],
                                            op0=ALU.mult, op1=ALU.add)
                    emit(2, pk)
                    for j in range(2, nf + 1):
                        pn = plane.tile([128, SBC], hf, tag=ptag, bufs=pbufs)
                        eng = nc.gpsimd if j > 8 - n_pool_f else nc.vector
                        eng.tensor_tensor(out=pn[:, :], in0=pk[:, :],
                                          in1=cdup[:, :], op=ALU.mult)
                        emit(1 + j, pn)
                        pk = pn

                def phase_b_mm(s):
                    acc = psacc.tile([OUT, SBC], dt, tag="acc")
                    accs[s] = acc
                    for c, F in bdefer.pop(s):
                        mmc_on(acc, cw(c), F, first=(c == 2))

                def phase_b_wav(s, defer=False):
                    if skip_wav:
                        wdefer.setdefault(s, [])
                        return
                    xsl = xds[s][:, :]
                    if defer:
                        chunks = wdefer.setdefault(s, [])
                        emit = lambda c, F: chunks.append((c, F))
                    else:
                        acc = accs[s]
                        emit = lambda c, F: mmc_on(acc, cw(c), F)
                    ptag = "pl3w" if defer else "pl"

                    for p in range(2):
                        u2 = cdwp.tile([128, SBC], hf, tag="u2", bufs=6)
                        nc.scalar.activation(u2[:, :], xsl, AF.Square,
                                             bias=wvt[:, 2 * p + 1:2 * p + 2],
                                             scale=wvt[:, 2 * p:2 * p + 1])
                        ew = cdwp.tile([128, SBC], hf, tag="ew", bufs=6)
                        nc.scalar.activation(ew[:, :], u2[:, :], AF.Exp,
                                             scale=-0.5)
                        wf = plane.tile([128, SBC], hf, tag=ptag,
                                        bufs=(2 if defer else 8))
                        nc.vector.tensor_tensor(out=wf[:, :], in0=u2[:, :],
                                                in1=ew[:, :], op=ALU.mult)
                        emit(10 + p, wf)
                        emit(12 + p, ew)

                def phase_b_wav_mm(s):
                    acc = accs[s]
                    for c, F in wdefer.pop(s):
                        mmc_on(acc, cw(c), F)

                # =========== phase C: post-collective remix ================
                def phase_c(gm):
                    v = pers.tile([128, 24], dt, tag="vecs")
                    rng_, msk = v[:, 2:3], v[:, 3:4]
                    gmn2, gmax2, rng2 = v[:, 4:5], v[:, 5:6], v[:, 6:7]
                    rinv, bb, aa = v[:, 7:8], v[:, 8:9], v[:, 9:10]
                    b2, b3, a2, a3 = v[:, 10:11], v[:, 11:12], v[:, 12:13], v[:, 13:14]
                    ab, a2b, ab2, rstep = (v[:, 14:15], v[:, 15:16], v[:, 16:17],
                                           v[:, 17:18])
                    # gm[:,0] = -gmin, gm[:,1] = gmax (already reduced)
                    gmn, gmax = gm[:, 0:1], gm[:, 1:2]
                    # rng = gmax + gmn  (gmn = -gmin)
                    nc.vector.tensor_tensor(out=rng_[:, :], in0=gmax[:, :],
                                            in1=gmn[:, :], op=ALU.add)
                    nc.vector.tensor_scalar(out=msk[:, :], in0=rng_[:, :],
                                            scalar1=1e-8, scalar2=0.5,
                                            op0=ALU.is_lt, op1=ALU.mult)
                    # gmn2 = -gmin2 = gmn + msk ;  gmax2 = gmax + msk
                    nc.vector.tensor_tensor(out=gmn2[:, :], in0=gmn[:, :],
                                            in1=msk[:, :], op=ALU.add)
                    nc.vector.tensor_tensor(out=gmax2[:, :], in0=gmax[:, :],
                                            in1=msk[:, :], op=ALU.add)
                    nc.vector.tensor_tensor(out=rng2[:, :], in0=gmax2[:, :],
                                            in1=gmn2[:, :], op=ALU.add)
                    nc.vector.reciprocal(out=rinv[:, :], in_=rng2[:, :])
                    nc.vector.tensor_scalar_mul(out=bb[:, :], in0=rinv[:, :],
                                                scalar1=8.0)
                    # aa = -gmin*b = gmn2*bb
                    nc.vector.tensor_tensor(out=aa[:, :], in0=gmn2[:, :],
                                            in1=bb[:, :], op=ALU.mult)
                    nc.vector.tensor_tensor(out=b2[:, :], in0=bb[:, :], in1=bb[:, :],
                                            op=ALU.mult)
                    nc.vector.tensor_tensor(out=b3[:, :], in0=b2[:, :], in1=bb[:, :],
                                            op=ALU.mult)
                    nc.vector.tensor_tensor(out=a2[:, :], in0=aa[:, :], in1=aa[:, :],
                                            op=ALU.mult)
                    nc.vector.tensor_tensor(out=a3[:, :], in0=a2[:, :], in1=aa[:, :],
                                            op=ALU.mult)
                    nc.vector.tensor_tensor(out=ab[:, :], in0=aa[:, :], in1=bb[:, :],
                                            op=ALU.mult)
                    nc.vector.tensor_tensor(out=a2b[:, :], in0=a2[:, :], in1=bb[:, :],
                                            op=ALU.mult)
                    nc.vector.tensor_tensor(out=ab2[:, :], in0=aa[:, :], in1=b2[:, :],
                                            op=ALU.mult)
                    nc.vector.tensor_scalar_mul(out=rstep[:, :], in0=rng2[:, :],
                                                scalar1=0.125)

                    # kappa_j = gmin2 + j*rng/8  ->  kp = -kappa = gmn2 - j*rstep
                    kn = pers.tile([128, 7], dt, tag="kn")
                    for j in range(1, 8):
                        nc.vector.scalar_tensor_tensor(
                            out=kn[:, j - 1:j], in0=rstep[:, :], scalar=-float(j),
                            in1=gmn2[:, :], op0=ALU.mult, op1=ALU.add)
                    kp = pers.tile([128, 4], dt, tag="kp")
                    for q in range(4):
                        nc.vector.tensor_copy(out=kp[0:IN, q:q + 1],
                                              in_=kn[0:IN, 2 * q:2 * q + 1])
                        if 2 * q + 1 < 7:
                            nc.vector.tensor_copy(out=kp[IN:128, q:q + 1],
                                                  in_=kn[IN:128, 2 * q + 1:2 * q + 2])

                    # dynamic monomial chunk coefs (for P and Q planes)
                    cdpf = pers.tile([128, OUT], dt, tag="cdpf")
                    cdqf = pers.tile([64, OUT], dt, tag="cdqf")
                    cd0 = pers.tile([128, OUT], dt, tag="cd0")
                    tmp = pers.tile([128, OUT], dt, tag="cdtmp")
                    P0, P1 = pwt[:, 0:32], pwt[:, 32:64]
                    P2, P3 = pwt[:, 64:96], pwt[:, 96:128]
                    # cd0 = P0 + a*P1 + a^2*P2 + a^3*P3   (const plane)
                    nc.vector.tensor_scalar(out=cd0[:, :], in0=P1, scalar1=aa[:, 0:1],
                                            scalar2=None, op0=ALU.mult)
                    nc.vector.tensor_tensor(out=cd0[:, :], in0=cd0[:, :], in1=P0,
                                            op=ALU.add)
                    nc.vector.tensor_scalar(out=tmp[:, :], in0=P2, scalar1=a2[:, 0:1],
                                            scalar2=None, op0=ALU.mult)
                    nc.vector.tensor_tensor(out=cd0[:, :], in0=cd0[:, :],
                                            in1=tmp[:, :], op=ALU.add)
                    nc.vector.tensor_scalar(out=tmp[:, :], in0=P3, scalar1=a3[:, 0:1],
                                            scalar2=None, op0=ALU.mult)
                    nc.vector.tensor_tensor(out=cd0[:, :], in0=cd0[:, :],
                                            in1=tmp[:, :], op=ALU.add)
                    # cdp top = b*P1 + 2ab*P2 + 3a^2b*P3  (x coef)
                    nc.vector.tensor_scalar(out=cdpf[:, :], in0=P1, scalar1=bb[:, 0:1],
                                            scalar2=None, op0=ALU.mult)
                    nc.vector.tensor_scalar(out=tmp[:, :], in0=P2, scalar1=ab[:, 0:1],
                                            scalar2=2.0, op0=ALU.mult, op1=ALU.mult)
                    nc.vector.tensor_tensor(out=cdpf[:, :], in0=cdpf[:, :],
                                            in1=tmp[:, :], op=ALU.add)
                    nc.vector.tensor_scalar(out=tmp[:, :], in0=P3, scalar1=a2b[:, 0:1],
                                            scalar2=3.0, op0=ALU.mult, op1=ALU.mult)
                    nc.vector.tensor_tensor(out=cdpf[:, :], in0=cdpf[:, :],
                                            in1=tmp[:, :], op=ALU.add)
                    # cdp bottom = b^3*P3  (x^3 coef)
                    nc.vector.tensor_scalar(out=cdpf[64:128, :], in0=P3[64:128, :],
                                            scalar1=b3[64:128, 0:1],
                                            scalar2=None, op0=ALU.mult)
                    # cdq (64 rows) = b^2*P2 + 3ab^2*P3  (x^2 coef)
                    nc.vector.tensor_scalar(out=cdqf[:, :], in0=P2[0:64, :],
                                            scalar1=b2[0:64, 0:1],
                                            scalar2=None, op0=ALU.mult)
                    nc.vector.tensor_scalar(out=tmp[0:64, :], in0=P3[0:64, :],
                                            scalar1=ab2[0:64, 0:1],
                                            scalar2=3.0, op0=ALU.mult, op1=ALU.mult)
                    nc.vector.tensor_tensor(out=cdqf[:, :], in0=cdqf[:, :],
                                            in1=tmp[0:64, :], op=ALU.add)
                    cwpd = pers.tile([128, OUT], hf, tag="cwpd")
                    nc.vector.tensor_tensor(out=cwpd[:, :], in0=cwf[:, 0:32],
                                            in1=cdpf[:, :], op=ALU.add)
                    cwqd = pers.tile([128, OUT], hf, tag="cwqd")
                    nc.vector.tensor_copy(out=cwqd[64:128, :],
                                          in_=cwf[64:128, 32:64])
                    nc.vector.tensor_tensor(out=cwqd[0:64, :],
                                            in0=cwf[0:64, 32:64],
                                            in1=cdqf[:, :], op=ALU.add)

                    # rho chunk coefs (fp16): cols 0..3 top/bottom = RW[j]*b^3;
                    # col3 bottom = bias rows (cot + cd0 sum plane)
                    rhw = pers.tile([128, 4 * OUT], hf, tag="rhw")
                    for j in range(1, 8):
                        q, half = (j - 1) // 2, (j - 1) % 2
                        r0, r1 = half * 64, (half + 1) * 64
                        nc.vector.tensor_scalar(
                            out=rhw[r0:r1, 32 * q:32 * (q + 1)],
                            in0=rwt[r0:r1, 32 * (j - 1):32 * j],
                            scalar1=b3[r0:r1, 0:1], scalar2=None, op0=ALU.mult)
                    cot2 = pers.tile([IN, OUT], dt, tag="cot2")
                    nc.vector.tensor_tensor(out=cot2[:, :], in0=cot[:, :],
                                            in1=cd0[0:64, :], op=ALU.add)
                    nc.vector.tensor_copy(out=rhw[64:128, 96:128], in_=cot2[:, :])
                    return kp, rhw, cwpd, cwqd

                # =========== phase D per super (rho + dyn chunks + out) ====
                def phase_d(s, kp, rhw, cwpd, cwqd):
                    xsl = xds[s][:, :]
                    acc = accs.pop(s)
                    for q in range([] if skip_rho else 4) if False else range(0 if skip_rho else 4):
                        if q < 3:
                            rq = plane.tile([128, SBC], hf, tag="pl")
                            nc.vector._custom_dve(OPS["ANT_FK_RELUCUBE"],
                                                  out=rq[:, :], in0=xsl,
                                                  s0=kp[:, q:q + 1])
                        else:
                            rq = rq3s[s]
                            nc.vector._custom_dve(OPS["ANT_FK_RELUCUBE"],
                                                  out=rq[0:64, :],
                                                  in0=xsl[0:64, :],
                                                  s0=kp[0:64, q:q + 1])
                        nc.tensor.matmul(acc[:, 0:BC], rhw[:, 32 * q:32 * (q + 1)],
                                         rq[:, 0:BC], start=False, stop=False)
                        nc.tensor.matmul(acc[:, BC:SBC], rhw[:, 32 * q:32 * (q + 1)],
                                         rq[:, BC:SBC], start=False, stop=False)
                    Pp = Ps.pop(s)
                    nc.tensor.matmul(acc[:, 0:BC], cwpd[:, :], Pp[:, 0:BC],
                                     start=False, stop=False)
                    nc.tensor.matmul(acc[:, BC:SBC], cwpd[:, :], Pp[:, BC:SBC],
                                     start=False, stop=False)
                    Qp = Qs.pop(s)
                    nc.tensor.matmul(acc[:, 0:BC], cwqd[:, :], Qp[:, 0:BC],
                                     start=False, stop=True)
                    nc.tensor.matmul(acc[:, BC:SBC], cwqd[:, :], Qp[:, BC:SBC],
                                     start=False, stop=True)
                    # undo the phase-A batch permutation (col t*128+p ->
                    # batch 8p+t) inside the PSUM->SBUF copy's read AP
                    yt = ytp.tile([OUT, SBC], dt, tag="yt")
                    nc.scalar.copy(
                        yt.rearrange("o (p g t) -> o p g t", p=128, g=2, t=4),
                        acc.rearrange("o (g t p) -> o p g t", g=2, t=4,
                                      p=128))
                    nc.sync.dma_start(out=y_d[:, s * SBC:(s + 1) * SBC],
                                      in_=yt[:, :])

                if phases == "A0":
                    ydum = ytp.tile([OUT, SBC], dt, tag="yt")
                    nc.vector.memset(ydum[:, :], 0.0)
                    nc.vector.tensor_copy(out=ydum[:, 0:NSUP],
                                          in_=mm[0:OUT, :])
                    for s in range(NSUP):
                        nc.sync.dma_start(out=y_d[:, s * SBC:(s + 1) * SBC],
                                          in_=ydum[:, :])
                    if _rep + 1 < reps:
                        phase_a_all()
                    continue
                gm2 = phase_coll()
                if phases == "A":
                    ydum = ytp.tile([OUT, SBC], dt, tag="yt")
                    nc.vector.memset(ydum[:, :], 0.0)
                    nc.vector.tensor_copy(out=ydum[:, 0:16], in_=gm2[0:OUT, :])
                    for s in range(NSUP):
                        nc.sync.dma_start(out=y_d[:, s * SBC:(s + 1) * SBC],
                                          in_=ydum[:, :])
                    if _rep + 1 < reps:
                        phase_a_all()
                    continue
                phase_b(0)
                phase_b(1)
                phase_b(2)
                phase_b(3, defer=True)
                phase_b_wav(0)
                phase_b_wav(1)
                phase_b_wav(2)
                phase_b_wav(3, defer=True)
                if _rep + 1 < reps:
                    phase_a_all()
                dyn = phase_c(gm2)
                phase_d(0, *dyn)
                phase_b_mm(3)
                phase_b_wav_mm(3)
                phase_d(1, *dyn)
                phase_d(2, *dyn)
                phase_d(3, *dyn)
    nc.compile()
    return nc


_NC_CACHE = None


def _get_nc():
    global _NC_CACHE
    if _NC_CACHE is None:
        _NC_CACHE = build_nc()
    return _NC_CACHE


def make_xs(x_shard):
    xdup = np.concatenate([np.asarray(x_shard, np.float16)] * 2, axis=1)
    xh = xdup.reshape(NSUP, 128, 8, 2 * IN).transpose(1, 0, 2, 3)
    return np.ascontiguousarray(xh).reshape(128, NSUP * 8 * 2 * IN)


def make_in_maps(inputs):
    consts = fold_constants(inputs)
    x = np.asarray(inputs["x"], np.float16)
    xdup = np.concatenate([x, x], axis=1)          # (B, 128)
    # per core: [128, (s, t, i)] with xs[p, s, t, i] = xdup[s*1024+8p+t, i]
    xh = xdup.reshape(N_CORES, NSUP, 128, 8, 2 * IN)
    xh = np.ascontiguousarray(xh.transpose(0, 2, 1, 3, 4)).reshape(
        N_CORES, 128, NSUP * 8 * 2 * IN)
    in_maps = []
    for c in range(N_CORES):
        m = {"xs": xh[c]}
        m.update(consts)
        in_maps.append(m)
    return in_maps


def kernel(**inputs) -> np.ndarray:
    from concourse.bass_utils import run_bass_kernel_spmd
    nc = _get_nc()
    in_maps = make_in_maps(inputs)
    res = run_bass_kernel_spmd(nc, in_maps, core_ids=list(range(N_CORES)))
    out = np.concatenate([res.results[c]["y"].T for c in range(N_CORES)], axis=0)
    return np.ascontiguousarray(out, dtype=F32)
